# revision 31
# baseline (speedup 1.0000x reference)
"""Trainium2 Bass kernel for nn_NodeEdgeBlock (gnn_message_passing).

Computes, for x:[512,256], e:[512,512,128]:
  Q,K,V = x@W{q,k,v}.T + b      (reshaped to heads; here kept flat [512,256])
  Y  = (Q_i * K_j)/sqrt(df) * (e_ij@Wem.T + bem + 1) + (e_ij@Wea.T + bea)
  newE = Y @ Weo.T + beo
  attn = softmax_j(Y); newX = (sum_j attn*V_j) @ Wxo.T + bxo

Sharding: query axis (dim 0) split across 8 cores, 64 queries each.

Per-core dataflow (features-on-partitions layout, [f, j] tiles):
  - e_i loaded natural [j,c] bf16 (SWDGE cast), PE-transposed to eT [c,j]
  - E1|E2 = wt12.T @ eT           (PE, PSUM f32)
  - G    = a*(E1) + a*(1+bem)     (ACT Identity w/ per-partition scale+bias)
  - Y1   = G * K.T                (DVE tt bf16)
  - Yf   = E2 += I.T@Y1           (PE identity-accumulate into E2's PSUM)
  - expY = Exp(Yf + bea), r1 = sum_j expY      (ACT w/ accum_out)
  - r2   = sum_j expY*V.T         (DVE tensor_tensor_reduce)
  - newE = eT.T@(Wea.T@Weo.T) + Y1.T@Weo.T + beo   (PE, 12 matmuls/query)
  - newX = (r2/r1) @ Wxo.T + bxo  (once at the end)
"""

import math

import numpy as np
import ml_dtypes

N = 512
XDIM = 256
EDIM = 128
DF = 32
NCORES = 8
QPC = N // NCORES  # 64 queries per core
SQ = math.sqrt(DF)

F32 = np.float32
BF16 = ml_dtypes.bfloat16

_prog_cache = {}


def _build_program(qpc):
    import concourse.bass as bass
    import concourse.mybir as mybir
    import concourse.tile as tile
    from concourse import bacc
    from concourse.masks import make_identity

    f32 = mybir.dt.float32
    bf16 = mybir.dt.bfloat16
    AF = mybir.ActivationFunctionType
    OP = mybir.AluOpType

    nc = bacc.Bacc("TRN2", target_bir_lowering=False, debug=False)

    # ---- DRAM I/O ----
    eb = nc.dram_tensor("eb", [qpc * N, EDIM], f32, kind="ExternalInput")
    xT_d = nc.dram_tensor("xT", [XDIM, N], f32, kind="ExternalInput")
    # per-core slice of x.T holding this core's own queries (for Q columns)
    xTq_d = nc.dram_tensor("xTq", [XDIM, qpc], f32, kind="ExternalInput")
    wqT_d = nc.dram_tensor("wqT", [XDIM, XDIM], bf16, kind="ExternalInput")
    wkT_d = nc.dram_tensor("wkT", [XDIM, XDIM], bf16, kind="ExternalInput")
    wvT_d = nc.dram_tensor("wvT", [XDIM, XDIM], bf16, kind="ExternalInput")
    wt12_d = nc.dram_tensor("wt12", [EDIM, 2 * XDIM], bf16, kind="ExternalInput")
    weoT_d = nc.dram_tensor("weoT", [XDIM, EDIM], bf16, kind="ExternalInput")
    waeo_d = nc.dram_tensor("waeo", [EDIM, EDIM], bf16, kind="ExternalInput")
    wxoT_d = nc.dram_tensor("wxoT", [XDIM, XDIM], bf16, kind="ExternalInput")
    bq_d = nc.dram_tensor("bq", [XDIM], f32, kind="ExternalInput")
    bk_d = nc.dram_tensor("bk", [XDIM], f32, kind="ExternalInput")
    bv_d = nc.dram_tensor("bv", [XDIM], f32, kind="ExternalInput")
    bem1s_d = nc.dram_tensor("bem1s", [XDIM], f32, kind="ExternalInput")  # (1+bem)/sq
    bea_d = nc.dram_tensor("bea", [XDIM], f32, kind="ExternalInput")
    beo_d = nc.dram_tensor("beo", [EDIM], f32, kind="ExternalInput")
    bxo_d = nc.dram_tensor("bxo", [XDIM], f32, kind="ExternalInput")

    ne_out = nc.dram_tensor("ne_out", [qpc * N, EDIM], f32, kind="ExternalOutput")
    nx_out = nc.dram_tensor("nx_out", [qpc, XDIM], f32, kind="ExternalOutput")

    with tile.TileContext(nc) as tc:
        with (
            tc.tile_pool(name="persist", bufs=1) as P,
            tc.tile_pool(name="work", bufs=3) as W,
            tc.tile_pool(name="ps", bufs=2, space="PSUM") as PS,
        ):
            # ---- constants / weights into SBUF ----
            ident = P.tile([128, 128], bf16)
            make_identity(nc, ident)

            wt12_sb = P.tile([128, 2 * XDIM], bf16)
            nc.sync.dma_start(out=wt12_sb, in_=wt12_d[:, :])
            weot_sb = P.tile([128, 2, EDIM], bf16)
            nc.sync.dma_start(
                out=weot_sb, in_=weoT_d[:, :].rearrange("(fc p) e -> p fc e", p=128)
            )
            waeo_sb = P.tile([128, EDIM], bf16)
            nc.sync.dma_start(out=waeo_sb, in_=waeo_d[:, :])
            wxot_sb = P.tile([128, 2, XDIM], bf16)
            nc.sync.dma_start(
                out=wxot_sb, in_=wxoT_d[:, :].rearrange("(fc p) o -> p fc o", p=128)
            )

            def col2(d, tag):  # [256] dram -> [128, 2] sbuf f32
                t = P.tile([128, 2], f32, tag=tag)
                nc.sync.dma_start(out=t, in_=d[:].rearrange("(t p) -> p t", p=128))
                return t

            bq_sb, bk_sb, bv_sb = (
                col2(bq_d, "bq_sb"),
                col2(bk_d, "bk_sb"),
                col2(bv_d, "bv_sb"),
            )
            bem1s_sb, bea_sb = col2(bem1s_d, "bem1s_sb"), col2(bea_d, "bea_sb")

            beo_sb = P.tile([128, 4, EDIM], f32)  # beo broadcast along j-partitions
            beo_ap = beo_d[:]
            nc.gpsimd.dma_start(
                out=beo_sb,
                in_=bass.AP(
                    tensor=beo_ap.tensor,
                    offset=beo_ap.offset,
                    ap=[[0, 128], [0, 4], [1, EDIM]],
                ),
            )
            bxo_sb = P.tile([qpc, XDIM], f32)  # bxo broadcast along i-partitions
            bxo_ap = bxo_d[:]
            nc.gpsimd.dma_start(
                out=bxo_sb,
                in_=bass.AP(
                    tensor=bxo_ap.tensor,
                    offset=bxo_ap.offset,
                    ap=[[0, qpc], [1, XDIM]],
                ),
            )

            # x.T cast to bf16; W{q,k,v}.T loaded [in-chunk, f]
            xt_sb = P.tile([128, 2, N], bf16)
            nc.gpsimd.dma_start(
                out=xt_sb, in_=xT_d[:, :].rearrange("(kc p) i -> p kc i", p=128)
            )
            xtq_sb = P.tile([128, 2, qpc], bf16)
            nc.gpsimd.dma_start(
                out=xtq_sb, in_=xTq_d[:, :].rearrange("(kc p) i -> p kc i", p=128)
            )
            wq_sb = P.tile([128, 2, XDIM], bf16)
            nc.sync.dma_start(
                out=wq_sb, in_=wqT_d[:, :].rearrange("(kc p) f -> p kc f", p=128)
            )
            wk_sb = P.tile([128, 2, XDIM], bf16)
            nc.sync.dma_start(
                out=wk_sb, in_=wkT_d[:, :].rearrange("(kc p) f -> p kc f", p=128)
            )
            wv_sb = P.tile([128, 2, XDIM], bf16)
            nc.sync.dma_start(
                out=wv_sb, in_=wvT_d[:, :].rearrange("(kc p) f -> p kc f", p=128)
            )

            # ---- persistent per-core tensors ----
            QTs = P.tile([128, 2, qpc], f32)  # (Q.T+bq)/sq          (a columns)
            A2s = P.tile([128, 2, qpc], f32)  # (Q.T+bq)*(1+bem)/sq  (bias columns)
            KTs = P.tile([128, 2, N], bf16)
            VTs = P.tile([128, 2, N], bf16)
            R1 = P.tile([128, 2, qpc], f32)
            R2 = P.tile([128, 2, qpc], f32)
            # per-query a-scaled Wem.T: aW[:, fo, q, :] = Wem.T[:, fo-half] * a[q, fo-half]
            aW = P.tile([128, 2, qpc, 128], bf16)

            # ---- Q/K/V projections: *.T = W*.T.T @ x.T ----
            for w_sb, kind in ((wq_sb, "q"), (wk_sb, "k"), (wv_sb, "v")):
                rhs_sb = xtq_sb if kind == "q" else xt_sb
                nq = qpc if kind == "q" else N
                for fo in range(2):
                    ps = PS.tile([128, nq], f32, tag="E1p")
                    for kc in range(2):
                        nc.tensor.matmul(
                            ps,
                            lhsT=w_sb[:, kc, fo * 128 : (fo + 1) * 128],
                            rhs=rhs_sb[:, kc, :],
                            start=(kc == 0),
                            stop=(kc == 1),
                        )
                    if kind == "q":
                        nc.vector.tensor_scalar(
                            QTs[:, fo, :], ps, bq_sb[:, fo : fo + 1], 1.0 / SQ,
                            OP.add, OP.mult,
                        )
                        nc.vector.tensor_scalar(
                            A2s[:, fo, :], ps, bq_sb[:, fo : fo + 1],
                            bem1s_sb[:, fo : fo + 1], OP.add, OP.mult,
                        )
                    elif kind == "k":
                        nc.vector.tensor_scalar(
                            KTs[:, fo, :], ps, bk_sb[:, fo : fo + 1], None, OP.add
                        )
                    else:
                        nc.vector.tensor_scalar(
                            VTs[:, fo, :], ps, bv_sb[:, fo : fo + 1], None, OP.add
                        )

            # ---- build aW (per-query scaled E1 weights) ----
            nfq = 2 * qpc  # (fo, q) pairs
            total = nfq * 128
            ascr_d = nc.dram_tensor("ascr", [total], bf16, kind="Internal")
            with tc.tile_pool(name="scratch", bufs=1) as SC:
                # (fo,q)-major row layout of the a-values via PE transpose
                qtp = PS.tile([128, 128], f32, tag="E2p")
                identf = SC.tile([128, 128], f32, tag="identf")
                make_identity(nc, identf)
                nc.tensor.transpose(
                    qtp[:nfq, :], QTs.rearrange("p a b -> p (a b)"), identf
                )
                qtr = SC.tile([nfq, 128], bf16, tag="qtr")
                nc.vector.tensor_copy(qtr, qtp[:nfq, :])
                # flatten partitions -> one row [1, (fo, q, f)] via DRAM roundtrip
                nc.sync.dma_start(
                    out=ascr_d[:].rearrange("(p f) -> p f", p=nfq), in_=qtr
                )
                aflat = SC.tile([1, total], bf16, tag="aflat")
                nc.sync.dma_start(
                    out=aflat, in_=ascr_d[:].rearrange("(o t) -> o t", o=1)
                )
                # replicate Wem.T columns per query: wrep[(fo, q, f)] on 128 c-parts
                wrep = SC.tile([128, 2, qpc, 128], bf16, tag="wrep")
                for fo in range(2):
                    wsrc = wt12_sb[:, fo * 128 : (fo + 1) * 128]
                    nc.gpsimd.dma_start(
                        out=wrep[:, fo],
                        in_=bass.AP(
                            tensor=wsrc.tensor,
                            offset=wsrc.offset,
                            ap=[wsrc.ap[0], [0, qpc], wsrc.ap[1]],
                        ),
                    )
                ones_row = SC.tile([1, 128], bf16, tag="ones_row")
                nc.vector.memset(ones_row, 1.0)
                chsz = min(512, total)
                aW_f = aW.rearrange("p a b c -> p (a b c)")
                wrep_f = wrep.rearrange("p a b c -> p (a b c)")
                for ch in range(total // chsz):
                    ap_ps = PS.tile([128, chsz], f32, tag="E1p")
                    nc.tensor.matmul(
                        ap_ps,
                        lhsT=ones_row,
                        rhs=aflat[:, ch * chsz : (ch + 1) * chsz],
                        start=True,
                        stop=True,
                    )
                    nc.vector.scalar_tensor_tensor(
                        out=aW_f[:, ch * chsz : (ch + 1) * chsz],
                        in0=ap_ps,
                        scalar=0.0,
                        in1=wrep_f[:, ch * chsz : (ch + 1) * chsz],
                        op0=OP.add,
                        op1=OP.mult,
                    )

            # ---- main loop (1-query software skew) ----
            state = {}

            def front(q):
                if q % 2 == 0:  # load e for two queries per DMA
                    enat2 = W.tile([128, 2, 4, EDIM], bf16, tag="enat")
                    nc.gpsimd.dma_start(
                        out=enat2,
                        in_=eb[q * N : (q + 2) * N, :].rearrange(
                            "(u t p) c -> p u t c", p=128, u=2
                        ),
                    )
                    state["enat"] = enat2
                enat = state["enat"][:, q % 2]
                eTp = PS.tile([128, 4, EDIM], bf16, tag="eTp")
                for t in range(4):
                    nc.tensor.transpose(eTp[:, t, :], enat[:, t, :], ident)
                eT = W.tile([128, 4, EDIM], bf16, tag="eT")
                nc.scalar.copy(eT, eTp)

                E1p = [PS.tile([128, N], f32, tag="E1p", name=f"E1p{fo}") for fo in range(2)]
                E2p = [PS.tile([128, N], f32, tag="E2p", name=f"E2p{fo}") for fo in range(2)]
                for fo in range(2):
                    # a[q] * E1 directly via pre-scaled weights
                    nc.tensor.matmul(
                        E1p[fo],
                        lhsT=aW[:, fo, q, :],
                        rhs=eT,
                        start=True,
                        stop=True,
                    )
                for fo in range(2):
                    nc.tensor.matmul(
                        E2p[fo],
                        lhsT=wt12_sb[:, XDIM + fo * 128 : XDIM + (fo + 1) * 128],
                        rhs=eT,
                        start=True,
                        stop=False,
                        skip_group_check=True,
                    )
                state[q] = (eT, E1p, E2p)

            def back(q):
                eT, E1p, E2p = state.pop(q)
                Y1 = W.tile([128, 2, N], bf16, tag="Y1")
                for fo in range(2):
                    # Y1 = (a*E1 + a2) * K.T   (fused: per-partition scalar add)
                    nc.vector.scalar_tensor_tensor(
                        out=Y1[:, fo, :],
                        in0=E1p[fo],
                        scalar=A2s[:, fo, q : q + 1],
                        in1=KTs[:, fo, :],
                        op0=OP.add,
                        op1=OP.mult,
                    )
                for fo in range(2):
                    nc.tensor.matmul(
                        E2p[fo],
                        lhsT=ident,
                        rhs=Y1[:, fo, :],
                        start=False,
                        stop=True,
                        skip_group_check=True,
                    )
                expY = W.tile([128, 2, N], bf16, tag="expY")
                Xscr = W.tile([128, N], bf16, tag="Xscr")
                for fo in range(2):
                    nc.scalar.activation(
                        expY[:, fo, :],
                        E2p[fo],
                        AF.Exp,
                        bias=bea_sb[:, fo : fo + 1],
                        scale=1.0,
                        accum_out=R1[:, fo, q : q + 1],
                    )
                    nc.vector.scalar_tensor_tensor(
                        out=Xscr,
                        in0=expY[:, fo, :],
                        scalar=1.0,
                        in1=VTs[:, fo, :],
                        op0=OP.mult,
                        op1=OP.mult,
                        accum_out=R2[:, fo, q : q + 1],
                    )
                nep = PS.tile([128, 4, EDIM], f32, tag="nep")
                for t in range(4):
                    nc.tensor.matmul(
                        nep[:, t, :], lhsT=eT[:, t, :], rhs=waeo_sb,
                        start=True, stop=False, skip_group_check=True,
                    )
                    nc.tensor.matmul(
                        nep[:, t, :],
                        lhsT=Y1[:, 0, t * 128 : (t + 1) * 128],
                        rhs=weot_sb[:, 0, :],
                        start=False, stop=False, skip_group_check=True,
                    )
                    nc.tensor.matmul(
                        nep[:, t, :],
                        lhsT=Y1[:, 1, t * 128 : (t + 1) * 128],
                        rhs=weot_sb[:, 1, :],
                        start=False, stop=True, skip_group_check=True,
                    )
                ne_sb = W.tile([128, 4, EDIM], f32, tag="ne_sb")
                nc.vector.scalar_tensor_tensor(
                    out=ne_sb, in0=nep, scalar=1.0, in1=beo_sb,
                    op0=OP.mult, op1=OP.add,
                )
                nc.sync.dma_start(
                    out=ne_out[q * N : (q + 1) * N, :].rearrange(
                        "(t p) c -> p t c", p=128
                    ),
                    in_=ne_sb,
                )

            for q in range(qpc):
                front(q)
                if q > 0:
                    back(q - 1)
            back(qpc - 1)

            # ---- finale: newX ----
            Rr = P.tile([128, 2, qpc], f32)
            nc.vector.reciprocal(Rr, R1)
            WVt = P.tile([128, 2, qpc], bf16)
            nc.vector.tensor_mul(WVt, R2, Rr)
            nxp = PS.tile([qpc, XDIM], f32, tag="nep")
            for fc in range(2):
                nc.tensor.matmul(
                    nxp, lhsT=WVt[:, fc, :], rhs=wxot_sb[:, fc, :],
                    start=(fc == 0), stop=(fc == 1),
                )
            nx_sb = P.tile([qpc, XDIM], f32)
            nc.vector.scalar_tensor_tensor(
                out=nx_sb, in0=nxp, scalar=1.0, in1=bxo_sb, op0=OP.mult, op1=OP.add
            )
            nc.sync.dma_start(out=nx_out[:, :], in_=nx_sb)

    nc.compile()
    return nc


def _get_program(qpc=QPC):
    if qpc not in _prog_cache:
        _prog_cache[qpc] = _build_program(qpc)
    return _prog_cache[qpc]


def _host_prep(x, Wq, bq, Wk, bk, Wv, bv, Wem, bem, Wea, bea, Wxo, bxo, Weo, beo):
    def b(a):
        return np.ascontiguousarray(a, dtype=np.float64).astype(BF16)

    common = {
        "xT": np.ascontiguousarray(x.T, dtype=F32),
        "wqT": b(Wq.T),
        "wkT": b(Wk.T),
        "wvT": b(Wv.T),
        "wt12": b(np.concatenate([Wem.T, Wea.T], axis=1)),
        "weoT": b(Weo.T),
        "waeo": (Wea.T.astype(np.float64) @ Weo.T.astype(np.float64)).astype(BF16),
        "wxoT": b(Wxo.T),
        "bq": np.ascontiguousarray(bq, F32),
        "bk": np.ascontiguousarray(bk, F32),
        "bv": np.ascontiguousarray(bv, F32),
        "bem1s": ((1.0 + bem.astype(np.float64)) / SQ).astype(F32),
        "bea": np.ascontiguousarray(bea, F32),
        # newE = Yf@Weo.T + beo with Yf = Y1 + E2 + bea; the device computes
        # eT.T@waeo + Y1.T@weoT (no bea), so fold bea@Weo.T into beo here.
        "beo": (
            beo.astype(np.float64) + bea.astype(np.float64) @ Weo.T.astype(np.float64)
        ).astype(F32),
        "bxo": np.ascontiguousarray(bxo, F32),
    }
    return common


def kernel(
    x, e, adj, Wq, bq, Wk, bk, Wv, bv, Wem, bem, Wea, bea, Wxo, bxo, Weo, beo,
    _trace=False,
):
    from concourse.bass_utils import run_bass_kernel_spmd

    x = np.asarray(x, F32)
    e = np.asarray(e, F32)
    common = _host_prep(
        x, Wq, bq, Wk, bk, Wv, bv, Wem, bem, Wea, bea, Wxo, bxo, Weo, beo
    )
    ef = e.reshape(N * N, EDIM)
    xT = common["xT"]
    in_maps = []
    for c in range(NCORES):
        m = dict(common)
        m["eb"] = ef[c * QPC * N : (c + 1) * QPC * N, :]
        m["xTq"] = np.ascontiguousarray(xT[:, c * QPC : (c + 1) * QPC])
        in_maps.append(m)

    nc = _get_program(QPC)
    if _trace:
        results = _run_traced(nc, in_maps)
    else:
        res = run_bass_kernel_spmd(nc, in_maps, core_ids=list(range(NCORES)))
        results = res.results
    newX = np.concatenate([results[c]["nx_out"] for c in range(NCORES)], axis=0)
    newE = np.concatenate(
        [results[c]["ne_out"].reshape(QPC, N, EDIM) for c in range(NCORES)],
        axis=0,
    )
    return (newX, newE)


def _run_traced(nc, in_maps, outdir="/tmp/ntff_prof"):
    """Run via PJRT with NRT/NTFF profiling of device 0 (axon ctypes hook)."""
    import os
    import shutil

    from concourse import bass2jax
    from trn_agent_boot.trn_boot import _ntff_profile_via_ctypes

    shutil.rmtree(outdir, ignore_errors=True)
    os.makedirs(outdir, exist_ok=True)

    # capture the exact NEFF bytes the PJRT path executes
    orig_rename = bass2jax.rename_neff_tensors_and_patch_header

    def _patched(neff_path, mapping):
        data = orig_rename(neff_path, mapping)
        with open(f"{outdir}/exec.neff", "wb") as f:
            f.write(data)
        return data

    bass2jax.rename_neff_tensors_and_patch_header = _patched
    try:
        hook = _ntff_profile_via_ctypes("/opt/axon/libaxon_pjrt.so")
        with hook(outdir, [0]):
            results = bass2jax.run_bass_via_pjrt(nc, in_maps, n_cores=NCORES)
    finally:
        bass2jax.rename_neff_tensors_and_patch_header = orig_rename
    kernel._last_ntff_dir = outdir
    return results


# revision 32
# speedup vs baseline: 1.0711x; 1.0711x over previous
"""Trainium2 Bass kernel for nn_NodeEdgeBlock (gnn_message_passing).

Computes, for x:[512,256], e:[512,512,128]:
  Q,K,V = x@W{q,k,v}.T + b      (reshaped to heads; here kept flat [512,256])
  Y  = (Q_i * K_j)/sqrt(df) * (e_ij@Wem.T + bem + 1) + (e_ij@Wea.T + bea)
  newE = Y @ Weo.T + beo
  attn = softmax_j(Y); newX = (sum_j attn*V_j) @ Wxo.T + bxo

Sharding: query axis (dim 0) split across 8 cores, 64 queries each.

Per-core dataflow (features-on-partitions layout, [f, j] tiles):
  - e_i loaded natural [j,c] bf16 (SWDGE cast), PE-transposed to eT [c,j]
  - E1|E2 = wt12.T @ eT           (PE, PSUM f32)
  - G    = a*(E1) + a*(1+bem)     (ACT Identity w/ per-partition scale+bias)
  - Y1   = G * K.T                (DVE tt bf16)
  - Yf   = E2 += I.T@Y1           (PE identity-accumulate into E2's PSUM)
  - expY = Exp(Yf + bea), r1 = sum_j expY      (ACT w/ accum_out)
  - r2   = sum_j expY*V.T         (DVE tensor_tensor_reduce)
  - newE = eT.T@(Wea.T@Weo.T) + Y1.T@Weo.T + beo   (PE, 12 matmuls/query)
  - newX = (r2/r1) @ Wxo.T + bxo  (once at the end)
"""

import math

import numpy as np
import ml_dtypes

N = 512
XDIM = 256
EDIM = 128
DF = 32
NCORES = 8
QPC = N // NCORES  # 64 queries per core
SQ = math.sqrt(DF)

F32 = np.float32
BF16 = ml_dtypes.bfloat16

_prog_cache = {}


def _build_program(qpc, beo0=False, bxo0=False):
    import concourse.bass as bass
    import concourse.mybir as mybir
    import concourse.tile as tile
    from concourse import bacc
    from concourse.masks import make_identity

    f32 = mybir.dt.float32
    bf16 = mybir.dt.bfloat16
    AF = mybir.ActivationFunctionType
    OP = mybir.AluOpType

    nc = bacc.Bacc("TRN2", target_bir_lowering=False, debug=False)

    # ---- DRAM I/O ----
    eb = nc.dram_tensor("eb", [qpc * N, EDIM], f32, kind="ExternalInput")
    xT_d = nc.dram_tensor("xT", [XDIM, N], f32, kind="ExternalInput")
    # per-core slice of x.T holding this core's own queries (for Q columns)
    xTq_d = nc.dram_tensor("xTq", [XDIM, qpc], f32, kind="ExternalInput")
    wqT_d = nc.dram_tensor("wqT", [XDIM, XDIM], bf16, kind="ExternalInput")
    wkT_d = nc.dram_tensor("wkT", [XDIM, XDIM], bf16, kind="ExternalInput")
    wvT_d = nc.dram_tensor("wvT", [XDIM, XDIM], bf16, kind="ExternalInput")
    wt12_d = nc.dram_tensor("wt12", [EDIM, 2 * XDIM], bf16, kind="ExternalInput")
    weoT_d = nc.dram_tensor("weoT", [XDIM, EDIM], bf16, kind="ExternalInput")
    waeo_d = nc.dram_tensor("waeo", [EDIM, EDIM], bf16, kind="ExternalInput")
    wxoT_d = nc.dram_tensor("wxoT", [XDIM, XDIM], bf16, kind="ExternalInput")
    bq_d = nc.dram_tensor("bq", [XDIM], f32, kind="ExternalInput")
    bk_d = nc.dram_tensor("bk", [XDIM], f32, kind="ExternalInput")
    bv_d = nc.dram_tensor("bv", [XDIM], f32, kind="ExternalInput")
    bem1s_d = nc.dram_tensor("bem1s", [XDIM], f32, kind="ExternalInput")  # (1+bem)/sq
    bea_d = nc.dram_tensor("bea", [XDIM], f32, kind="ExternalInput")
    beo_d = nc.dram_tensor("beo", [EDIM], f32, kind="ExternalInput")
    bxo_d = nc.dram_tensor("bxo", [XDIM], f32, kind="ExternalInput")

    ne_out = nc.dram_tensor("ne_out", [qpc * N, EDIM], f32, kind="ExternalOutput")
    nx_out = nc.dram_tensor("nx_out", [qpc, XDIM], f32, kind="ExternalOutput")

    with tile.TileContext(nc) as tc:
        with (
            tc.tile_pool(name="persist", bufs=1) as P,
            tc.tile_pool(name="work", bufs=3) as W,
            tc.tile_pool(name="ps", bufs=2, space="PSUM") as PS,
        ):
            # ---- constants / weights into SBUF ----
            ident = P.tile([128, 128], bf16)
            make_identity(nc, ident)

            wt12_sb = P.tile([128, 2 * XDIM], bf16)
            nc.sync.dma_start(out=wt12_sb, in_=wt12_d[:, :])
            weot_sb = P.tile([128, 2, EDIM], bf16)
            nc.sync.dma_start(
                out=weot_sb, in_=weoT_d[:, :].rearrange("(fc p) e -> p fc e", p=128)
            )
            waeo_sb = P.tile([128, EDIM], bf16)
            nc.sync.dma_start(out=waeo_sb, in_=waeo_d[:, :])
            wxot_sb = P.tile([128, 2, XDIM], bf16)
            nc.sync.dma_start(
                out=wxot_sb, in_=wxoT_d[:, :].rearrange("(fc p) o -> p fc o", p=128)
            )

            def col2(d, tag):  # [256] dram -> [128, 2] sbuf f32
                t = P.tile([128, 2], f32, tag=tag)
                nc.sync.dma_start(out=t, in_=d[:].rearrange("(t p) -> p t", p=128))
                return t

            bq_sb, bk_sb, bv_sb = (
                col2(bq_d, "bq_sb"),
                col2(bk_d, "bk_sb"),
                col2(bv_d, "bv_sb"),
            )
            bem1s_sb, bea_sb = col2(bem1s_d, "bem1s_sb"), col2(bea_d, "bea_sb")

            beo_sb = P.tile([128, 4, EDIM], f32)  # beo broadcast along j-partitions
            beo_ap = beo_d[:]
            nc.gpsimd.dma_start(
                out=beo_sb,
                in_=bass.AP(
                    tensor=beo_ap.tensor,
                    offset=beo_ap.offset,
                    ap=[[0, 128], [0, 4], [1, EDIM]],
                ),
            )
            bxo_sb = P.tile([qpc, XDIM], f32)  # bxo broadcast along i-partitions
            bxo_ap = bxo_d[:]
            nc.gpsimd.dma_start(
                out=bxo_sb,
                in_=bass.AP(
                    tensor=bxo_ap.tensor,
                    offset=bxo_ap.offset,
                    ap=[[0, qpc], [1, XDIM]],
                ),
            )

            # x.T cast to bf16; W{q,k,v}.T loaded [in-chunk, f]
            xt_sb = P.tile([128, 2, N], bf16)
            nc.gpsimd.dma_start(
                out=xt_sb, in_=xT_d[:, :].rearrange("(kc p) i -> p kc i", p=128)
            )
            xtq_sb = P.tile([128, 2, qpc], bf16)
            nc.gpsimd.dma_start(
                out=xtq_sb, in_=xTq_d[:, :].rearrange("(kc p) i -> p kc i", p=128)
            )
            wq_sb = P.tile([128, 2, XDIM], bf16)
            nc.sync.dma_start(
                out=wq_sb, in_=wqT_d[:, :].rearrange("(kc p) f -> p kc f", p=128)
            )
            wk_sb = P.tile([128, 2, XDIM], bf16)
            nc.sync.dma_start(
                out=wk_sb, in_=wkT_d[:, :].rearrange("(kc p) f -> p kc f", p=128)
            )
            wv_sb = P.tile([128, 2, XDIM], bf16)
            nc.sync.dma_start(
                out=wv_sb, in_=wvT_d[:, :].rearrange("(kc p) f -> p kc f", p=128)
            )

            # ---- persistent per-core tensors ----
            QTs = P.tile([128, 2, qpc], f32)  # (Q.T+bq)/sq          (a columns)
            A2s = P.tile([128, 2, qpc], f32)  # (Q.T+bq)*(1+bem)/sq  (bias columns)
            KTs = P.tile([128, 2, N], bf16)
            VTs = P.tile([128, 2, N], bf16)
            R1 = P.tile([128, 2, qpc], f32)
            R2 = P.tile([128, 2, qpc], f32)
            # per-query a-scaled Wem.T: aW[:, fo, q, :] = Wem.T[:, fo-half] * a[q, fo-half]
            aW = P.tile([128, 2, qpc, 128], bf16)

            # ---- Q/K/V projections: *.T = W*.T.T @ x.T ----
            for w_sb, kind in ((wq_sb, "q"), (wk_sb, "k"), (wv_sb, "v")):
                rhs_sb = xtq_sb if kind == "q" else xt_sb
                nq = qpc if kind == "q" else N
                for fo in range(2):
                    ps = PS.tile([128, nq], f32, tag="E1p")
                    for kc in range(2):
                        nc.tensor.matmul(
                            ps,
                            lhsT=w_sb[:, kc, fo * 128 : (fo + 1) * 128],
                            rhs=rhs_sb[:, kc, :],
                            start=(kc == 0),
                            stop=(kc == 1),
                        )
                    if kind == "q":
                        nc.vector.tensor_scalar(
                            QTs[:, fo, :], ps, bq_sb[:, fo : fo + 1], 1.0 / SQ,
                            OP.add, OP.mult,
                        )
                        nc.vector.tensor_scalar(
                            A2s[:, fo, :], ps, bq_sb[:, fo : fo + 1],
                            bem1s_sb[:, fo : fo + 1], OP.add, OP.mult,
                        )
                    elif kind == "k":
                        nc.vector.tensor_scalar(
                            KTs[:, fo, :], ps, bk_sb[:, fo : fo + 1], None, OP.add
                        )
                    else:
                        nc.vector.tensor_scalar(
                            VTs[:, fo, :], ps, bv_sb[:, fo : fo + 1], None, OP.add
                        )

            # ---- build aW (per-query scaled E1 weights) ----
            nfq = 2 * qpc  # (fo, q) pairs
            total = nfq * 128
            ascr_d = nc.dram_tensor("ascr", [total], bf16, kind="Internal")
            with tc.tile_pool(name="scratch", bufs=1) as SC:
                # (fo,q)-major row layout of the a-values via PE transpose
                qtp = PS.tile([128, 128], f32, tag="E2p")
                identf = SC.tile([128, 128], f32, tag="identf")
                make_identity(nc, identf)
                nc.tensor.transpose(
                    qtp[:nfq, :], QTs.rearrange("p a b -> p (a b)"), identf
                )
                qtr = SC.tile([nfq, 128], bf16, tag="qtr")
                nc.vector.tensor_copy(qtr, qtp[:nfq, :])
                # flatten partitions -> one row [1, (fo, q, f)] via DRAM roundtrip
                nc.sync.dma_start(
                    out=ascr_d[:].rearrange("(p f) -> p f", p=nfq), in_=qtr
                )
                aflat = SC.tile([1, total], bf16, tag="aflat")
                nc.sync.dma_start(
                    out=aflat, in_=ascr_d[:].rearrange("(o t) -> o t", o=1)
                )
                # replicate Wem.T columns per query: wrep[(fo, q, f)] on 128 c-parts
                wrep = SC.tile([128, 2, qpc, 128], bf16, tag="wrep")
                for fo in range(2):
                    wsrc = wt12_sb[:, fo * 128 : (fo + 1) * 128]
                    nc.gpsimd.dma_start(
                        out=wrep[:, fo],
                        in_=bass.AP(
                            tensor=wsrc.tensor,
                            offset=wsrc.offset,
                            ap=[wsrc.ap[0], [0, qpc], wsrc.ap[1]],
                        ),
                    )
                ones_row = SC.tile([1, 128], bf16, tag="ones_row")
                nc.vector.memset(ones_row, 1.0)
                chsz = min(512, total)
                aW_f = aW.rearrange("p a b c -> p (a b c)")
                wrep_f = wrep.rearrange("p a b c -> p (a b c)")
                for ch in range(total // chsz):
                    ap_ps = PS.tile([128, chsz], f32, tag="E1p")
                    nc.tensor.matmul(
                        ap_ps,
                        lhsT=ones_row,
                        rhs=aflat[:, ch * chsz : (ch + 1) * chsz],
                        start=True,
                        stop=True,
                    )
                    nc.vector.scalar_tensor_tensor(
                        out=aW_f[:, ch * chsz : (ch + 1) * chsz],
                        in0=ap_ps,
                        scalar=0.0,
                        in1=wrep_f[:, ch * chsz : (ch + 1) * chsz],
                        op0=OP.add,
                        op1=OP.mult,
                    )

            # ---- main loop (1-query software skew) ----
            state = {}

            def front(q):
                if q % 2 == 0:  # load e for two queries per DMA
                    enat2 = W.tile([128, 2, 4, EDIM], bf16, tag="enat")
                    nc.gpsimd.dma_start(
                        out=enat2,
                        in_=eb[q * N : (q + 2) * N, :].rearrange(
                            "(u t p) c -> p u t c", p=128, u=2
                        ),
                    )
                    state["enat"] = enat2
                enat = state["enat"][:, q % 2]
                eTp = PS.tile([128, 4, EDIM], bf16, tag="eTp")
                for t in range(4):
                    nc.tensor.transpose(eTp[:, t, :], enat[:, t, :], ident)
                eT = W.tile([128, 4, EDIM], bf16, tag="eT")
                nc.scalar.copy(eT, eTp)

                E1p = [PS.tile([128, N], f32, tag="E1p", name=f"E1p{fo}") for fo in range(2)]
                E2p = [PS.tile([128, N], f32, tag="E2p", name=f"E2p{fo}") for fo in range(2)]
                for fo in range(2):
                    # a[q] * E1 directly via pre-scaled weights
                    nc.tensor.matmul(
                        E1p[fo],
                        lhsT=aW[:, fo, q, :],
                        rhs=eT,
                        start=True,
                        stop=True,
                    )
                for fo in range(2):
                    nc.tensor.matmul(
                        E2p[fo],
                        lhsT=wt12_sb[:, XDIM + fo * 128 : XDIM + (fo + 1) * 128],
                        rhs=eT,
                        start=True,
                        stop=False,
                        skip_group_check=True,
                    )
                state[q] = (eT, E1p, E2p)

            def back(q):
                eT, E1p, E2p = state.pop(q)
                Y1 = W.tile([128, 2, N], bf16, tag="Y1")
                for fo in range(2):
                    # Y1 = (a*E1 + a2) * K.T   (fused: per-partition scalar add)
                    nc.vector.scalar_tensor_tensor(
                        out=Y1[:, fo, :],
                        in0=E1p[fo],
                        scalar=A2s[:, fo, q : q + 1],
                        in1=KTs[:, fo, :],
                        op0=OP.add,
                        op1=OP.mult,
                    )
                for fo in range(2):
                    nc.tensor.matmul(
                        E2p[fo],
                        lhsT=ident,
                        rhs=Y1[:, fo, :],
                        start=False,
                        stop=True,
                        skip_group_check=True,
                    )
                expY = W.tile([128, 2, N], bf16, tag="expY")
                Xscr = W.tile([128, N], bf16, tag="Xscr")
                for fo in range(2):
                    nc.scalar.activation(
                        expY[:, fo, :],
                        E2p[fo],
                        AF.Exp,
                        bias=bea_sb[:, fo : fo + 1],
                        scale=1.0,
                        accum_out=R1[:, fo, q : q + 1],
                    )
                    nc.vector.scalar_tensor_tensor(
                        out=Xscr,
                        in0=expY[:, fo, :],
                        scalar=1.0,
                        in1=VTs[:, fo, :],
                        op0=OP.mult,
                        op1=OP.mult,
                        accum_out=R2[:, fo, q : q + 1],
                    )
                nep = PS.tile([128, 4, EDIM], f32, tag="nep")
                for t in range(4):
                    nc.tensor.matmul(
                        nep[:, t, :], lhsT=eT[:, t, :], rhs=waeo_sb,
                        start=True, stop=False, skip_group_check=True,
                    )
                    nc.tensor.matmul(
                        nep[:, t, :],
                        lhsT=Y1[:, 0, t * 128 : (t + 1) * 128],
                        rhs=weot_sb[:, 0, :],
                        start=False, stop=False, skip_group_check=True,
                    )
                    nc.tensor.matmul(
                        nep[:, t, :],
                        lhsT=Y1[:, 1, t * 128 : (t + 1) * 128],
                        rhs=weot_sb[:, 1, :],
                        start=False, stop=True, skip_group_check=True,
                    )
                ne_sb = W.tile([128, 4, EDIM], f32, tag="ne_sb")
                if beo0:
                    nc.scalar.copy(ne_sb, nep)
                else:
                    nc.vector.scalar_tensor_tensor(
                        out=ne_sb, in0=nep, scalar=1.0, in1=beo_sb,
                        op0=OP.mult, op1=OP.add,
                    )
                nc.sync.dma_start(
                    out=ne_out[q * N : (q + 1) * N, :].rearrange(
                        "(t p) c -> p t c", p=128
                    ),
                    in_=ne_sb,
                )

            for q in range(qpc):
                front(q)
                if q > 0:
                    back(q - 1)
            back(qpc - 1)

            # ---- finale: newX ----
            Rr = P.tile([128, 2, qpc], f32)
            nc.vector.reciprocal(Rr, R1)
            WVt = P.tile([128, 2, qpc], bf16)
            nc.vector.tensor_mul(WVt, R2, Rr)
            nxp = PS.tile([qpc, XDIM], f32, tag="nep")
            for fc in range(2):
                nc.tensor.matmul(
                    nxp, lhsT=WVt[:, fc, :], rhs=wxot_sb[:, fc, :],
                    start=(fc == 0), stop=(fc == 1),
                )
            nx_sb = P.tile([qpc, XDIM], f32)
            if bxo0:
                nc.vector.tensor_scalar(nx_sb, nxp, 0.0, None, OP.add)
            else:
                nc.vector.scalar_tensor_tensor(
                    out=nx_sb, in0=nxp, scalar=1.0, in1=bxo_sb, op0=OP.mult, op1=OP.add
                )
            nc.sync.dma_start(out=nx_out[:, :], in_=nx_sb)

    nc.compile()
    return nc


def _get_program(qpc=QPC, beo0=False, bxo0=False):
    key = (qpc, beo0, bxo0)
    if key not in _prog_cache:
        _prog_cache[key] = _build_program(qpc, beo0, bxo0)
    return _prog_cache[key]


def _host_prep(x, Wq, bq, Wk, bk, Wv, bv, Wem, bem, Wea, bea, Wxo, bxo, Weo, beo):
    def b(a):
        return np.ascontiguousarray(a, dtype=np.float64).astype(BF16)

    common = {
        "xT": np.ascontiguousarray(x.T, dtype=F32),
        "wqT": b(Wq.T),
        "wkT": b(Wk.T),
        "wvT": b(Wv.T),
        "wt12": b(np.concatenate([Wem.T, Wea.T], axis=1)),
        "weoT": b(Weo.T),
        "waeo": (Wea.T.astype(np.float64) @ Weo.T.astype(np.float64)).astype(BF16),
        "wxoT": b(Wxo.T),
        "bq": np.ascontiguousarray(bq, F32),
        "bk": np.ascontiguousarray(bk, F32),
        "bv": np.ascontiguousarray(bv, F32),
        "bem1s": ((1.0 + bem.astype(np.float64)) / SQ).astype(F32),
        "bea": np.ascontiguousarray(bea, F32),
        # newE = Yf@Weo.T + beo with Yf = Y1 + E2 + bea; the device computes
        # eT.T@waeo + Y1.T@weoT (no bea), so fold bea@Weo.T into beo here.
        "beo": (
            beo.astype(np.float64) + bea.astype(np.float64) @ Weo.T.astype(np.float64)
        ).astype(F32),
        "bxo": np.ascontiguousarray(bxo, F32),
    }
    return common


def kernel(
    x, e, adj, Wq, bq, Wk, bk, Wv, bv, Wem, bem, Wea, bea, Wxo, bxo, Weo, beo,
    _trace=False,
):
    from concourse.bass_utils import run_bass_kernel_spmd

    x = np.asarray(x, F32)
    e = np.asarray(e, F32)
    common = _host_prep(
        x, Wq, bq, Wk, bk, Wv, bv, Wem, bem, Wea, bea, Wxo, bxo, Weo, beo
    )
    ef = e.reshape(N * N, EDIM)
    xT = common["xT"]
    in_maps = []
    for c in range(NCORES):
        m = dict(common)
        m["eb"] = ef[c * QPC * N : (c + 1) * QPC * N, :]
        m["xTq"] = np.ascontiguousarray(xT[:, c * QPC : (c + 1) * QPC])
        in_maps.append(m)

    beo0 = not np.any(common["beo"])
    bxo0 = not np.any(common["bxo"])
    nc = _get_program(QPC, beo0, bxo0)
    if _trace:
        results = _run_traced(nc, in_maps)
    else:
        res = run_bass_kernel_spmd(nc, in_maps, core_ids=list(range(NCORES)))
        results = res.results
    newX = np.concatenate([results[c]["nx_out"] for c in range(NCORES)], axis=0)
    newE = np.concatenate(
        [results[c]["ne_out"].reshape(QPC, N, EDIM) for c in range(NCORES)],
        axis=0,
    )
    return (newX, newE)


def _run_traced(nc, in_maps, outdir="/tmp/ntff_prof"):
    """Run via PJRT with NRT/NTFF profiling of device 0 (axon ctypes hook)."""
    import os
    import shutil

    from concourse import bass2jax
    from trn_agent_boot.trn_boot import _ntff_profile_via_ctypes

    shutil.rmtree(outdir, ignore_errors=True)
    os.makedirs(outdir, exist_ok=True)

    # capture the exact NEFF bytes the PJRT path executes
    orig_rename = bass2jax.rename_neff_tensors_and_patch_header

    def _patched(neff_path, mapping):
        data = orig_rename(neff_path, mapping)
        with open(f"{outdir}/exec.neff", "wb") as f:
            f.write(data)
        return data

    bass2jax.rename_neff_tensors_and_patch_header = _patched
    try:
        hook = _ntff_profile_via_ctypes("/opt/axon/libaxon_pjrt.so")
        with hook(outdir, [0]):
            results = bass2jax.run_bass_via_pjrt(nc, in_maps, n_cores=NCORES)
    finally:
        bass2jax.rename_neff_tensors_and_patch_header = orig_rename
    kernel._last_ntff_dir = outdir
    return results


# revision 33
# speedup vs baseline: 1.1447x; 1.0686x over previous
"""Trainium2 Bass kernel for nn_NodeEdgeBlock (gnn_message_passing).

Computes, for x:[512,256], e:[512,512,128]:
  Q,K,V = x@W{q,k,v}.T + b      (reshaped to heads; here kept flat [512,256])
  Y  = (Q_i * K_j)/sqrt(df) * (e_ij@Wem.T + bem + 1) + (e_ij@Wea.T + bea)
  newE = Y @ Weo.T + beo
  attn = softmax_j(Y); newX = (sum_j attn*V_j) @ Wxo.T + bxo

Sharding: query axis (dim 0) split across 8 cores, 64 queries each.

Per-core dataflow (features-on-partitions layout, [f, j] tiles):
  - e_i loaded natural [j,c] bf16 (SWDGE cast), PE-transposed to eT [c,j]
  - E1|E2 = wt12.T @ eT           (PE, PSUM f32)
  - G    = a*(E1) + a*(1+bem)     (ACT Identity w/ per-partition scale+bias)
  - Y1   = G * K.T                (DVE tt bf16)
  - Yf   = E2 += I.T@Y1           (PE identity-accumulate into E2's PSUM)
  - expY = Exp(Yf + bea), r1 = sum_j expY      (ACT w/ accum_out)
  - r2   = sum_j expY*V.T         (DVE tensor_tensor_reduce)
  - newE = eT.T@(Wea.T@Weo.T) + Y1.T@Weo.T + beo   (PE, 12 matmuls/query)
  - newX = (r2/r1) @ Wxo.T + bxo  (once at the end)
"""

import math

import numpy as np
import ml_dtypes

N = 512
XDIM = 256
EDIM = 128
DF = 32
NCORES = 8
QPC = N // NCORES  # 64 queries per core
SQ = math.sqrt(DF)

F32 = np.float32
BF16 = ml_dtypes.bfloat16

_prog_cache = {}


def _build_program(qpc, beo0=False, bxo0=False):
    import concourse.bass as bass
    import concourse.mybir as mybir
    import concourse.tile as tile
    from concourse import bacc
    from concourse.masks import make_identity

    f32 = mybir.dt.float32
    bf16 = mybir.dt.bfloat16
    AF = mybir.ActivationFunctionType
    OP = mybir.AluOpType

    nc = bacc.Bacc("TRN2", target_bir_lowering=False, debug=False)

    # ---- DRAM I/O ----
    eb = nc.dram_tensor("eb", [qpc * N, EDIM], f32, kind="ExternalInput")
    xT_d = nc.dram_tensor("xT", [XDIM, N], f32, kind="ExternalInput")
    # per-core slice of x.T holding this core's own queries (for Q columns)
    xTq_d = nc.dram_tensor("xTq", [XDIM, qpc], f32, kind="ExternalInput")
    wqT_d = nc.dram_tensor("wqT", [XDIM, XDIM], bf16, kind="ExternalInput")
    wkT_d = nc.dram_tensor("wkT", [XDIM, XDIM], bf16, kind="ExternalInput")
    wvT_d = nc.dram_tensor("wvT", [XDIM, XDIM], bf16, kind="ExternalInput")
    wt12_d = nc.dram_tensor("wt12", [EDIM, 2 * XDIM], bf16, kind="ExternalInput")
    weoT_d = nc.dram_tensor("weoT", [XDIM, EDIM], bf16, kind="ExternalInput")
    waeo_d = nc.dram_tensor("waeo", [EDIM, EDIM], bf16, kind="ExternalInput")
    wxoT_d = nc.dram_tensor("wxoT", [XDIM, XDIM], bf16, kind="ExternalInput")
    bq_d = nc.dram_tensor("bq", [XDIM], f32, kind="ExternalInput")
    bk_d = nc.dram_tensor("bk", [XDIM], f32, kind="ExternalInput")
    bv_d = nc.dram_tensor("bv", [XDIM], f32, kind="ExternalInput")
    bem1s_d = nc.dram_tensor("bem1s", [XDIM], f32, kind="ExternalInput")  # (1+bem)/sq
    bea_d = nc.dram_tensor("bea", [XDIM], f32, kind="ExternalInput")
    beo_d = nc.dram_tensor("beo", [EDIM], f32, kind="ExternalInput")
    bxo_d = nc.dram_tensor("bxo", [XDIM], f32, kind="ExternalInput")

    ne_out = nc.dram_tensor("ne_out", [qpc * N, EDIM], f32, kind="ExternalOutput")
    nx_out = nc.dram_tensor("nx_out", [qpc, XDIM], f32, kind="ExternalOutput")

    with tile.TileContext(nc) as tc:
        with (
            tc.tile_pool(name="persist", bufs=1) as P,
            tc.tile_pool(name="work", bufs=3) as W,
            tc.tile_pool(name="ps", bufs=2, space="PSUM") as PS,
        ):
            # ---- constants / weights into SBUF ----
            ident = P.tile([128, 128], bf16)
            make_identity(nc, ident)

            wt12_sb = P.tile([128, 2 * XDIM], bf16)
            nc.sync.dma_start(out=wt12_sb, in_=wt12_d[:, :])
            weot_sb = P.tile([128, 2, EDIM], bf16)
            nc.sync.dma_start(
                out=weot_sb, in_=weoT_d[:, :].rearrange("(fc p) e -> p fc e", p=128)
            )
            waeo_sb = P.tile([128, EDIM], bf16)
            nc.sync.dma_start(out=waeo_sb, in_=waeo_d[:, :])
            wxot_sb = P.tile([128, 2, XDIM], bf16)
            nc.sync.dma_start(
                out=wxot_sb, in_=wxoT_d[:, :].rearrange("(fc p) o -> p fc o", p=128)
            )

            def col2(d, tag):  # [256] dram -> [128, 2] sbuf f32
                t = P.tile([128, 2], f32, tag=tag)
                nc.sync.dma_start(out=t, in_=d[:].rearrange("(t p) -> p t", p=128))
                return t

            bq_sb, bk_sb, bv_sb = (
                col2(bq_d, "bq_sb"),
                col2(bk_d, "bk_sb"),
                col2(bv_d, "bv_sb"),
            )
            bem1s_sb, bea_sb = col2(bem1s_d, "bem1s_sb"), col2(bea_d, "bea_sb")

            beo_sb = P.tile([128, 4, EDIM], f32)  # beo broadcast along j-partitions
            beo_ap = beo_d[:]
            nc.gpsimd.dma_start(
                out=beo_sb,
                in_=bass.AP(
                    tensor=beo_ap.tensor,
                    offset=beo_ap.offset,
                    ap=[[0, 128], [0, 4], [1, EDIM]],
                ),
            )
            bxo_sb = P.tile([qpc, XDIM], f32)  # bxo broadcast along i-partitions
            bxo_ap = bxo_d[:]
            nc.gpsimd.dma_start(
                out=bxo_sb,
                in_=bass.AP(
                    tensor=bxo_ap.tensor,
                    offset=bxo_ap.offset,
                    ap=[[0, qpc], [1, XDIM]],
                ),
            )

            # x.T cast to bf16; W{q,k,v}.T loaded [in-chunk, f]
            xt_sb = P.tile([128, 2, N], bf16)
            nc.gpsimd.dma_start(
                out=xt_sb, in_=xT_d[:, :].rearrange("(kc p) i -> p kc i", p=128)
            )
            xtq_sb = P.tile([128, 2, qpc], bf16)
            nc.gpsimd.dma_start(
                out=xtq_sb, in_=xTq_d[:, :].rearrange("(kc p) i -> p kc i", p=128)
            )
            wq_sb = P.tile([128, 2, XDIM], bf16)
            nc.sync.dma_start(
                out=wq_sb, in_=wqT_d[:, :].rearrange("(kc p) f -> p kc f", p=128)
            )
            wk_sb = P.tile([128, 2, XDIM], bf16)
            nc.sync.dma_start(
                out=wk_sb, in_=wkT_d[:, :].rearrange("(kc p) f -> p kc f", p=128)
            )
            wv_sb = P.tile([128, 2, XDIM], bf16)
            nc.sync.dma_start(
                out=wv_sb, in_=wvT_d[:, :].rearrange("(kc p) f -> p kc f", p=128)
            )

            # ---- persistent per-core tensors ----
            QTs = P.tile([128, 2, qpc], f32)  # (Q.T+bq)/sq          (a columns)
            A2s = P.tile([128, 2, qpc], f32)  # (Q.T+bq)*(1+bem)/sq  (bias columns)
            KTs = P.tile([128, 2, N], bf16)
            VTs = P.tile([128, 2, N], bf16)
            R1 = P.tile([128, 2, qpc], f32)
            R2 = P.tile([128, 2, qpc], f32)
            # per-query a-scaled Wem.T: aW[:, fo, q, :] = Wem.T[:, fo-half] * a[q, fo-half]
            aW = P.tile([128, 2, qpc, 128], bf16)

            # ---- Q/K/V projections: *.T = W*.T.T @ x.T ----
            for w_sb, kind in ((wq_sb, "q"), (wk_sb, "k"), (wv_sb, "v")):
                rhs_sb = xtq_sb if kind == "q" else xt_sb
                nq = qpc if kind == "q" else N
                for fo in range(2):
                    ps = PS.tile([128, nq], f32, tag="E1p")
                    for kc in range(2):
                        nc.tensor.matmul(
                            ps,
                            lhsT=w_sb[:, kc, fo * 128 : (fo + 1) * 128],
                            rhs=rhs_sb[:, kc, :],
                            start=(kc == 0),
                            stop=(kc == 1),
                        )
                    if kind == "q":
                        nc.vector.tensor_scalar(
                            QTs[:, fo, :], ps, bq_sb[:, fo : fo + 1], 1.0 / SQ,
                            OP.add, OP.mult,
                        )
                        nc.vector.tensor_scalar(
                            A2s[:, fo, :], ps, bq_sb[:, fo : fo + 1],
                            bem1s_sb[:, fo : fo + 1], OP.add, OP.mult,
                        )
                    elif kind == "k":
                        nc.vector.tensor_scalar(
                            KTs[:, fo, :], ps, bk_sb[:, fo : fo + 1], None, OP.add
                        )
                    else:
                        nc.vector.tensor_scalar(
                            VTs[:, fo, :], ps, bv_sb[:, fo : fo + 1], None, OP.add
                        )

            # ---- build aW (per-query scaled E1 weights) ----
            nfq = 2 * qpc  # (fo, q) pairs
            total = nfq * 128
            ascr_d = nc.dram_tensor("ascr", [total], bf16, kind="Internal")
            with tc.tile_pool(name="scratch", bufs=1) as SC:
                # (fo,q)-major row layout of the a-values via PE transpose
                qtp = PS.tile([128, 128], f32, tag="E2p")
                identf = SC.tile([128, 128], f32, tag="identf")
                make_identity(nc, identf)
                nc.tensor.transpose(
                    qtp[:nfq, :], QTs.rearrange("p a b -> p (a b)"), identf
                )
                qtr = SC.tile([nfq, 128], bf16, tag="qtr")
                nc.vector.tensor_copy(qtr, qtp[:nfq, :])
                # flatten partitions -> a row per core via DRAM roundtrip, then
                # broadcast back to all 128 c-partitions
                nc.sync.dma_start(
                    out=ascr_d[:].rearrange("(p f) -> p f", p=nfq), in_=qtr
                )
                A_sb = SC.tile([128, total], bf16, tag="A_sb")
                ascr_ap = ascr_d[:]
                nc.sync.dma_start(
                    out=A_sb,
                    in_=bass.AP(
                        tensor=ascr_ap.tensor,
                        offset=ascr_ap.offset,
                        ap=[[0, 128], [1, total]],
                    ),
                )
                # aW[(c, fo, q, f)] = A[(fo, q, f)] * Wem.T[c, (fo, f)]
                aW_v = aW.rearrange("p a b c -> p a b c")
                wsrc = wt12_sb[:, 0:XDIM].rearrange("p (a f) -> p a f", a=2)
                w_bc = bass.AP(
                    tensor=wsrc.tensor,
                    offset=wsrc.offset,
                    ap=[wsrc.ap[0], wsrc.ap[1], [0, qpc], wsrc.ap[2]],
                )
                nc.vector.tensor_mul(
                    aW_v, A_sb.rearrange("p (a b c) -> p a b c", a=2, b=qpc), w_bc
                )

            # ---- main loop (1-query software skew) ----
            state = {}

            def front(q):
                if q % 2 == 0:  # load e for two queries per DMA
                    enat2 = W.tile([128, 2, 4, EDIM], bf16, tag="enat")
                    nc.gpsimd.dma_start(
                        out=enat2,
                        in_=eb[q * N : (q + 2) * N, :].rearrange(
                            "(u t p) c -> p u t c", p=128, u=2
                        ),
                    )
                    state["enat"] = enat2
                enat = state["enat"][:, q % 2]
                eTp = PS.tile([128, 4, EDIM], bf16, tag="eTp")
                for t in range(4):
                    nc.tensor.transpose(eTp[:, t, :], enat[:, t, :], ident)
                eT = W.tile([128, 4, EDIM], bf16, tag="eT")
                nc.scalar.copy(eT, eTp)

                E1p = [PS.tile([128, N], f32, tag="E1p", name=f"E1p{fo}") for fo in range(2)]
                E2p = [PS.tile([128, N], f32, tag="E2p", name=f"E2p{fo}") for fo in range(2)]
                for fo in range(2):
                    # a[q] * E1 directly via pre-scaled weights
                    nc.tensor.matmul(
                        E1p[fo],
                        lhsT=aW[:, fo, q, :],
                        rhs=eT,
                        start=True,
                        stop=True,
                    )
                for fo in range(2):
                    nc.tensor.matmul(
                        E2p[fo],
                        lhsT=wt12_sb[:, XDIM + fo * 128 : XDIM + (fo + 1) * 128],
                        rhs=eT,
                        start=True,
                        stop=False,
                        skip_group_check=True,
                    )
                state[q] = (eT, E1p, E2p)

            def back(q):
                eT, E1p, E2p = state.pop(q)
                Y1 = W.tile([128, 2, N], bf16, tag="Y1")
                for fo in range(2):
                    # Y1 = (a*E1 + a2) * K.T   (fused: per-partition scalar add)
                    nc.vector.scalar_tensor_tensor(
                        out=Y1[:, fo, :],
                        in0=E1p[fo],
                        scalar=A2s[:, fo, q : q + 1],
                        in1=KTs[:, fo, :],
                        op0=OP.add,
                        op1=OP.mult,
                    )
                for fo in range(2):
                    nc.tensor.matmul(
                        E2p[fo],
                        lhsT=ident,
                        rhs=Y1[:, fo, :],
                        start=False,
                        stop=True,
                        skip_group_check=True,
                    )
                expY = W.tile([128, 2, N], bf16, tag="expY")
                Xscr = W.tile([128, N], bf16, tag="Xscr")
                for fo in range(2):
                    nc.scalar.activation(
                        expY[:, fo, :],
                        E2p[fo],
                        AF.Exp,
                        bias=bea_sb[:, fo : fo + 1],
                        scale=1.0,
                        accum_out=R1[:, fo, q : q + 1],
                    )
                    nc.vector.scalar_tensor_tensor(
                        out=Xscr,
                        in0=expY[:, fo, :],
                        scalar=1.0,
                        in1=VTs[:, fo, :],
                        op0=OP.mult,
                        op1=OP.mult,
                        accum_out=R2[:, fo, q : q + 1],
                    )
                nep = PS.tile([128, 4, EDIM], f32, tag="nep")
                for t in range(4):
                    nc.tensor.matmul(
                        nep[:, t, :], lhsT=eT[:, t, :], rhs=waeo_sb,
                        start=True, stop=False, skip_group_check=True,
                    )
                    nc.tensor.matmul(
                        nep[:, t, :],
                        lhsT=Y1[:, 0, t * 128 : (t + 1) * 128],
                        rhs=weot_sb[:, 0, :],
                        start=False, stop=False, skip_group_check=True,
                    )
                    nc.tensor.matmul(
                        nep[:, t, :],
                        lhsT=Y1[:, 1, t * 128 : (t + 1) * 128],
                        rhs=weot_sb[:, 1, :],
                        start=False, stop=True, skip_group_check=True,
                    )
                ne_sb = W.tile([128, 4, EDIM], f32, tag="ne_sb")
                if beo0:
                    nc.scalar.copy(ne_sb, nep)
                else:
                    nc.vector.scalar_tensor_tensor(
                        out=ne_sb, in0=nep, scalar=1.0, in1=beo_sb,
                        op0=OP.mult, op1=OP.add,
                    )
                nc.sync.dma_start(
                    out=ne_out[q * N : (q + 1) * N, :].rearrange(
                        "(t p) c -> p t c", p=128
                    ),
                    in_=ne_sb,
                )

            for q in range(qpc):
                front(q)
                if q > 0:
                    back(q - 1)
            back(qpc - 1)

            # ---- finale: newX ----
            Rr = P.tile([128, 2, qpc], f32)
            nc.vector.reciprocal(Rr, R1)
            WVt = P.tile([128, 2, qpc], bf16)
            nc.vector.tensor_mul(WVt, R2, Rr)
            nxp = PS.tile([qpc, XDIM], f32, tag="nep")
            for fc in range(2):
                nc.tensor.matmul(
                    nxp, lhsT=WVt[:, fc, :], rhs=wxot_sb[:, fc, :],
                    start=(fc == 0), stop=(fc == 1),
                )
            nx_sb = P.tile([qpc, XDIM], f32)
            if bxo0:
                nc.vector.tensor_scalar(nx_sb, nxp, 0.0, None, OP.add)
            else:
                nc.vector.scalar_tensor_tensor(
                    out=nx_sb, in0=nxp, scalar=1.0, in1=bxo_sb, op0=OP.mult, op1=OP.add
                )
            nc.sync.dma_start(out=nx_out[:, :], in_=nx_sb)

    nc.compile()
    return nc


def _get_program(qpc=QPC, beo0=False, bxo0=False):
    key = (qpc, beo0, bxo0)
    if key not in _prog_cache:
        _prog_cache[key] = _build_program(qpc, beo0, bxo0)
    return _prog_cache[key]


def _host_prep(x, Wq, bq, Wk, bk, Wv, bv, Wem, bem, Wea, bea, Wxo, bxo, Weo, beo):
    def b(a):
        return np.ascontiguousarray(a, dtype=np.float64).astype(BF16)

    common = {
        "xT": np.ascontiguousarray(x.T, dtype=F32),
        "wqT": b(Wq.T),
        "wkT": b(Wk.T),
        "wvT": b(Wv.T),
        "wt12": b(np.concatenate([Wem.T, Wea.T], axis=1)),
        "weoT": b(Weo.T),
        "waeo": (Wea.T.astype(np.float64) @ Weo.T.astype(np.float64)).astype(BF16),
        "wxoT": b(Wxo.T),
        "bq": np.ascontiguousarray(bq, F32),
        "bk": np.ascontiguousarray(bk, F32),
        "bv": np.ascontiguousarray(bv, F32),
        "bem1s": ((1.0 + bem.astype(np.float64)) / SQ).astype(F32),
        "bea": np.ascontiguousarray(bea, F32),
        # newE = Yf@Weo.T + beo with Yf = Y1 + E2 + bea; the device computes
        # eT.T@waeo + Y1.T@weoT (no bea), so fold bea@Weo.T into beo here.
        "beo": (
            beo.astype(np.float64) + bea.astype(np.float64) @ Weo.T.astype(np.float64)
        ).astype(F32),
        "bxo": np.ascontiguousarray(bxo, F32),
    }
    return common


def kernel(
    x, e, adj, Wq, bq, Wk, bk, Wv, bv, Wem, bem, Wea, bea, Wxo, bxo, Weo, beo,
    _trace=False,
):
    from concourse.bass_utils import run_bass_kernel_spmd

    x = np.asarray(x, F32)
    e = np.asarray(e, F32)
    common = _host_prep(
        x, Wq, bq, Wk, bk, Wv, bv, Wem, bem, Wea, bea, Wxo, bxo, Weo, beo
    )
    ef = e.reshape(N * N, EDIM)
    xT = common["xT"]
    in_maps = []
    for c in range(NCORES):
        m = dict(common)
        m["eb"] = ef[c * QPC * N : (c + 1) * QPC * N, :]
        m["xTq"] = np.ascontiguousarray(xT[:, c * QPC : (c + 1) * QPC])
        in_maps.append(m)

    beo0 = not np.any(common["beo"])
    bxo0 = not np.any(common["bxo"])
    nc = _get_program(QPC, beo0, bxo0)
    if _trace:
        results = _run_traced(nc, in_maps)
    else:
        res = run_bass_kernel_spmd(nc, in_maps, core_ids=list(range(NCORES)))
        results = res.results
    newX = np.concatenate([results[c]["nx_out"] for c in range(NCORES)], axis=0)
    newE = np.concatenate(
        [results[c]["ne_out"].reshape(QPC, N, EDIM) for c in range(NCORES)],
        axis=0,
    )
    return (newX, newE)


def _run_traced(nc, in_maps, outdir="/tmp/ntff_prof"):
    """Run via PJRT with NRT/NTFF profiling of device 0 (axon ctypes hook)."""
    import os
    import shutil

    from concourse import bass2jax
    from trn_agent_boot.trn_boot import _ntff_profile_via_ctypes

    shutil.rmtree(outdir, ignore_errors=True)
    os.makedirs(outdir, exist_ok=True)

    # capture the exact NEFF bytes the PJRT path executes
    orig_rename = bass2jax.rename_neff_tensors_and_patch_header

    def _patched(neff_path, mapping):
        data = orig_rename(neff_path, mapping)
        with open(f"{outdir}/exec.neff", "wb") as f:
            f.write(data)
        return data

    bass2jax.rename_neff_tensors_and_patch_header = _patched
    try:
        hook = _ntff_profile_via_ctypes("/opt/axon/libaxon_pjrt.so")
        with hook(outdir, [0]):
            results = bass2jax.run_bass_via_pjrt(nc, in_maps, n_cores=NCORES)
    finally:
        bass2jax.rename_neff_tensors_and_patch_header = orig_rename
    kernel._last_ntff_dir = outdir
    return results
